# revision 13
# baseline (speedup 1.0000x reference)
"""Causal multi-head attention on 8 TRN2 NeuronCores.

Sharding: core c = (batch b=c//2, head-group g=c%2). Each core computes QKV
projections + causal attention for its 8 heads over the full sequence of its
batch; 2-rank AllGathers (pair shares a batch) exchange attention outputs;
each core then runs the output projection for its half of the output columns.

v2 schedule notes:
  - host inputs re-laid-out so every DMA tile is one contiguous DRAM block;
    DMA waves ordered by first-use time (pair-0 qk weights + x chunk 0 first).
  - attention(0,0) starts right after qk(0,0)+v(0); remaining QKV chunks are
    PE fillers inside earlier attention chunks (global filler queue).
  - v_aug packed [P, 8, 65] (ones col + 64 v cols, no zero pad); softmax
    denominator comes from the ones row of the AV matmul (psum row 0,
    values rows 1:65).
  - exp pipeline: pending AV queue depth 5, 7 et buffers.
  - output projection: blocks [0,1,4,5,2,6] pre-accumulated into bf16 partial
    during pair-3 attention (phase A'); pair-3 attention outputs are gathered
    per q-chunk j; phase B(j) (blocks 3,7 + partial) pipelines behind each
    gather so only gather(3)+B(3) remain on the tail.
"""

import numpy as np
import ml_dtypes

import concourse.bass as bass
import concourse.mybir as mybir
import concourse.tile as tile
from concourse import bacc
from concourse import bass_utils

BF16 = mybir.dt.bfloat16
F32 = mybir.dt.float32

B, S, D = 4, 2048, 1024
H, DK = 16, 64
HPG = 8          # heads per group (per core)
DG = HPG * DK    # 512, d-range per core
NPAIR = 4        # head pairs per core
SC = 512         # sequence chunk (matmul free dim)
NSC = S // SC    # 4
KB = 128         # key block
NKB = S // KB    # 16
P = 128
NI = D // P      # 8
VW = DK + 1      # 65: ones col + v cols per head

_cache = {}


def _build():
    nc = bacc.Bacc("TRN2", target_bir_lowering=False, debug=False, num_devices=8)

    xTt = nc.dram_tensor("xTt", [NSC, P, NI, SC], BF16, kind="ExternalInput")
    wqt = nc.dram_tensor("wqt", [NPAIR, P, NI, P], BF16, kind="ExternalInput")
    wkt = nc.dram_tensor("wkt", [NPAIR, P, NI, P], BF16, kind="ExternalInput")
    wvt = nc.dram_tensor("wvt", [P, NI, DG], BF16, kind="ExternalInput")
    wot = nc.dram_tensor("wot", [P, NI, DG], BF16, kind="ExternalInput")
    bq = nc.dram_tensor("bq", [P, NPAIR], F32, kind="ExternalInput")
    bk = nc.dram_tensor("bk", [P, NPAIR], F32, kind="ExternalInput")
    bv_bc = nc.dram_tensor("bv_bc", [P, DG], F32, kind="ExternalInput")
    bo_bc = nc.dram_tensor("bo_bc", [P, DG], F32, kind="ExternalInput")
    masks = nc.dram_tensor("masks", [P, 4, SC], BF16, kind="ExternalInput")
    out = nc.dram_tensor("out", [S, DG], F32, kind="ExternalOutput")

    with tile.TileContext(nc) as tc:
        _emit(nc, tc, xTt, wqt, wkt, wvt, wot, bq, bk, bv_bc, bo_bc, masks, out)
    nc.compile()
    return nc


def _emit(nc, tc, xTt, wqt, wkt, wvt, wot, bq, bk, bv_bc, bo_bc, masks, out):
    ctxs = []

    def pool(name, bufs, space="SBUF"):
        cm = tc.tile_pool(name=name, bufs=bufs, space=space)
        p = cm.__enter__()
        ctxs.append(cm)
        return p

    const = pool("const", 1)
    dram = pool("dram", 1, space="DRAM")
    qk_pool = pool("qk", 2)
    att_pool = pool("att", 2)
    exp_pool = pool("exp", 5)
    small = pool("small", 3)
    out_pool = pool("outp", 3)
    ps_qk = pool("ps_qk", 2, space="PSUM")
    ps_sc = pool("ps_sc", 2, space="PSUM")
    ps_av = pool("ps_av", 2, space="PSUM")

    # ---- constants / weights, DMA waves ordered by first-use time ----
    xt = const.tile([P, NI, S], BF16, name="xt")
    wq = const.tile([P, NI, DG], BF16, name="wq")
    wk = const.tile([P, NI, DG], BF16, name="wk")
    wv = const.tile([P, NI, DG], BF16, name="wv")
    # wo aliases wv's storage (wv dead after the last v_chunk filler)
    wo = const.tile([P, NI, DG], BF16, tag="wv", name="wo")
    # wave 1: x chunk 0, pair-0 qk weight columns, small tensors
    for ii in range(0, NI, 2):
        nc.sync.dma_start(xt[:, ii:ii + 2, 0:SC], xTt[0, :, ii:ii + 2])
    nc.sync.dma_start(wq[:, :, 0:P], wqt[0])
    nc.sync.dma_start(wk[:, :, 0:P], wkt[0])
    bq_t = const.tile([P, NPAIR], F32, name="bq_t")
    bk_t = const.tile([P, NPAIR], F32, name="bk_t")
    bv_t = const.tile([P, DG], F32, name="bv_t")
    nc.sync.dma_start(bq_t[:], bq[:])
    nc.sync.dma_start(bk_t[:], bk[:])
    nc.sync.dma_start(bv_t[:], bv_bc[:])
    mask_t = const.tile([P, 4, SC], BF16, name="mask_t")
    nc.sync.dma_start(mask_t[:], masks[:])
    # wave 2: v weights (needed by v_chunk(0))
    nc.sync.dma_start(wv[:], wvt[:])
    # wave 3: remaining x chunks
    for sc in range(1, NSC):
        nc.sync.dma_start(xt[:, :, SC * sc:SC * (sc + 1)], xTt[sc])
    # wave 4: qk weights pairs 1-3
    for p in range(1, NPAIR):
        nc.sync.dma_start(wq[:, :, P * p:P * (p + 1)], wqt[p])
        nc.sync.dma_start(wk[:, :, P * p:P * (p + 1)], wkt[p])
    # wave 5: output projection weights + bias
    nc.sync.dma_start(wo[:], wot[:])
    bo_t = const.tile([P, DG], F32, name="bo_t")
    nc.sync.dma_start(bo_t[:], bo_bc[:])

    # v_aug[t]: [128, 8, 128]; per head h: col 0 = ones (softmax denominator
    # row), cols 1:64 = zeros (psum partition alignment pad), cols 64:128 = v.
    # Built via memsets (no host DMA).
    v_aug = [const.tile([P, HPG, P], BF16, name=f"va{t}", tag=f"va{t}")
             for t in range(NKB)]
    for t in range(NKB):
        nc.gpsimd.memset(v_aug[t][:, :, 0:DK], 0.0)
        nc.gpsimd.memset(v_aug[t][:, :, 0:1], 1.0)

    # DRAM bounce buffers for the pairwise AllGathers
    agin = dram.tile([3, P, S], BF16, name="agin")          # pairs 0-2
    agout0 = dram.tile([2, 2, P, S], BF16, name="agout0")   # pairs 0-1
    agout1 = dram.tile([2, 1, P, S], BF16, name="agout1")   # pair 2
    agin3 = dram.tile([NSC, P, SC], BF16, name="agin3")     # pair 3, per chunk
    agout3a = dram.tile([2, 2, P, SC], BF16, name="agout3a")  # chunks 0+1
    agout3b = dram.tile([2, P, SC], BF16, name="agout3b")     # chunk 2
    agout3c = dram.tile([2, P, SC], BF16, name="agout3c")     # chunk 3

    groups = [[0, 1], [2, 3], [4, 5], [6, 7]]

    qT_pair = [qk_pool.tile([P, S], BF16, tag="qT", name=f"qTp{pp}")
               for pp in range(NPAIR)]
    kT_pair = [qk_pool.tile([P, S], BF16, tag="kT", name=f"kTp{pp}")
               for pp in range(NPAIR)]

    def qk_chunk(p, sc):
        """q/k projections for pair p, seq chunk sc."""
        ssl = slice(SC * sc, SC * (sc + 1))
        ps_q = ps_qk.tile([P, SC], F32, tag="psqk", name=f"psq{p}_{sc}")
        for i in range(NI):
            nc.tensor.matmul(ps_q[:], lhsT=wq[:, i, P * p:P * (p + 1)],
                             rhs=xt[:, i, ssl], start=(i == 0), stop=(i == 7))
        nc.vector.tensor_add(qT_pair[p][:, ssl], ps_q[:],
                             bq_t[:, p:p + 1].to_broadcast((P, SC)))
        ps_k = ps_qk.tile([P, SC], F32, tag="psqk", name=f"psk{p}_{sc}")
        for i in range(NI):
            nc.tensor.matmul(ps_k[:], lhsT=wk[:, i, P * p:P * (p + 1)],
                             rhs=xt[:, i, ssl], start=(i == 0), stop=(i == 7))
        nc.vector.tensor_add(kT_pair[p][:, ssl], ps_k[:],
                             bk_t[:, p:p + 1].to_broadcast((P, SC)))

    def v_chunk(sc):
        """v projection for seq chunk sc (all 8 heads), into v_aug tiles."""
        for st in range(4):
            t = 4 * sc + st
            ps_v = ps_qk.tile([P, DG], F32, tag="psqk", name=f"psv{sc}_{st}")
            for i in range(NI):
                nc.tensor.matmul(ps_v[:], lhsT=xt[:, i, P * t:P * (t + 1)],
                                 rhs=wv[:, i, :], start=(i == 0), stop=(i == 7))
            va3 = v_aug[t]
            nc.vector.tensor_add(va3[:, :, DK:P],
                                 ps_v[:].rearrange("p (h c) -> p h c", c=DK),
                                 bv_t[:].rearrange("p (h c) -> p h c", c=DK))

    def attention_chunk(p, j, att, pop_filler):
        """Causal attention for head pair p, q chunk j. Both heads row-packed
        into one wide psum; one wide exp; diag blocks first."""
        avs = [ps_av.tile([P, SC], F32, tag="av", name=f"av{p}_{j}_{h}")
               for h in range(2)]
        nkb = 4 * (j + 1)
        kbs = list(range(4 * j, nkb)) + list(range(0, 4 * j))  # diag first
        pending = []
        issued = [0]

        def issue_av(item):
            kb, qlo, et = item
            et3 = et.rearrange("p (h w) -> p h w", w=SC)
            for h in range(2):
                hh = 2 * p + h
                nc.tensor.matmul(avs[h][:, qlo:], lhsT=v_aug[kb][:, hh, :],
                                 rhs=et3[:, h, qlo:],
                                 start=(issued[0] == 0),
                                 stop=(issued[0] == nkb - 1))
            issued[0] += 1

        for n, kb in enumerate(kbs):
            r = kb - 4 * j  # >= 0 on diagonal blocks
            qlo = P * r if r >= 0 else 0
            ps_s = ps_sc.tile([P, 2 * SC], F32, tag="sc", name=f"pss{p}_{j}_{kb}")
            for h in range(2):
                hb = slice(DK * h, DK * (h + 1))
                nc.tensor.matmul(
                    ps_s[:, SC * h + qlo:SC * (h + 1)],
                    lhsT=kT_pair[p][hb, P * kb:P * (kb + 1)],
                    rhs=qT_pair[p][hb, SC * j + qlo:SC * (j + 1)],
                    start=True, stop=True)
            et = exp_pool.tile([P, 2 * SC], BF16, tag="exp", name=f"et{p}_{j}_{kb}")
            ps3 = ps_s.rearrange("p (h w) -> p h w", w=SC)
            et3 = et.rearrange("p (h w) -> p h w", w=SC)
            nc.scalar.activation(et3[:, :, qlo:], ps3[:, :, qlo:],
                                 mybir.ActivationFunctionType.Exp, scale=0.125)
            if r >= 0:
                nc.vector.tensor_mul(
                    et3[:, :, qlo:], et3[:, :, qlo:],
                    mask_t[:, r:r + 1, qlo:].to_broadcast((P, 2, SC - qlo)))
            pending.append((kb, qlo, et))
            while len(pending) > 4:
                issue_av(pending.pop(0))
            if n % 2 == 1:
                pop_filler()
        while pending:
            issue_av(pending.pop(0))

        # eagerly free the av psum slots: reciprocal of the sums row plus a
        # copy of the value rows to SBUF; the gpsimd broadcast + multiply are
        # lazy and may be delayed by collectives on the gpsimd queue without
        # stalling the PSUM pipeline.
        sums_l = []
        avc_l = []
        for h in range(2):
            sums = small.tile([1, SC], F32, tag="sums", name=f"sums{p}_{j}_{h}")
            nc.vector.reciprocal_approx_fast(sums[0:1, :], avs[h][0:1, :])
            sums_l.append(sums)
            avc = small.tile([P, SC], F32, tag=f"avc{h}", name=f"avc{p}_{j}_{h}",
                             bufs=2)
            nc.vector.tensor_copy(avc[DK:P, :], avs[h][DK:P, :])
            avc_l.append(avc)

        def normalize():
            for h in range(2):
                rb = small.tile([P, SC], F32, tag="rb", name=f"rb{p}_{j}_{h}")
                nc.gpsimd.partition_broadcast(rb[:], sums_l[h][0:1, :])
                nc.vector.tensor_mul(att[h][DK:P, SC * j:SC * (j + 1)],
                                     avc_l[h][DK:P, :], rb[DK:P, :])
        return normalize

    # gathered attention outputs (out-proj lhsT), one [P, S] tile per i-block
    agt = [const.tile([P, S], BF16, name=f"agt{i}", tag=f"agt{i}")
           for i in range(NI)]
    # out-proj partials (blocks 0,1,4,5,2,6), bf16, bo included
    part_lo = const.tile([P, NI, SC], BF16, tag="wq", name="part_lo")
    part_hi = const.tile([P, NI, SC], BF16, tag="wk", name="part_hi")

    def agt_lhsT(i, qt):
        return agt[i][:, P * qt:P * (qt + 1)]

    def part_slice(qt):
        t = part_lo if qt < 8 else part_hi
        return t[:, qt % 8, :]

    def outproj_a(qt):
        """Phase A': accumulate blocks 0,1,4,5,2,6 for q-tile qt -> bf16
        partial (bias included)."""
        ps_o = ps_qk.tile([P, DG], F32, tag="psqk", name=f"psoa{qt}")
        for n, i in enumerate([0, 1, 4, 5, 2, 6]):
            nc.tensor.matmul(ps_o[:], lhsT=agt_lhsT(i, qt),
                             rhs=wo[:, i, :], start=(n == 0), stop=(n == 5))
        nc.vector.tensor_add(part_slice(qt), ps_o[:], bo_t[:])

    def phase_b(j):
        """Phase B(j): blocks 3,7 + phase-A partial -> out, for q-tiles of
        gather chunk j."""
        for qt in range(4 * j, 4 * (j + 1)):
            ps_o = ps_qk.tile([P, DG], F32, tag="psqk", name=f"psob{qt}")
            for n, i in enumerate([3, 7]):
                nc.tensor.matmul(ps_o[:], lhsT=agt_lhsT(i, qt),
                                 rhs=wo[:, i, :], start=(n == 0), stop=(n == 1))
            ot = out_pool.tile([P, DG], F32, tag="ot", name=f"ot{qt}")
            nc.vector.tensor_add(ot[:], ps_o[:], part_slice(qt))
            q = nc.sync if qt % 2 == 0 else nc.scalar
            q.dma_start(out[P * qt:P * (qt + 1), :], ot[:])

    def gather3(js, agout):
        """Pair-3 partial gather of chunks js (contiguous run) for both heads
        -> agt[3], agt[7] column slices."""
        for jj in js:
            jsl = slice(SC * jj, SC * (jj + 1))
            nc.sync.dma_start(agin3[jj, 0:DK], att3[0][DK:P, jsl])
            nc.sync.dma_start(agin3[jj, DK:P], att3[1][DK:P, jsl])
        nc.gpsimd.collective_compute(
            "AllGather", mybir.AluOpType.bypass, replica_groups=groups,
            ins=[agin3[js[0]:js[-1] + 1].opt()], outs=[agout[:].opt()])
        for n, jj in enumerate(js):
            jsl = slice(SC * jj, SC * (jj + 1))
            if len(js) > 1:
                nc.sync.dma_start(agt[3][:, jsl], agout[0, n])
                nc.sync.dma_start(agt[7][:, jsl], agout[1, n])
            else:
                nc.sync.dma_start(agt[3][:, jsl], agout[0])
                nc.sync.dma_start(agt[7][:, jsl], agout[1])

    # ---- global filler queue (PE work injected inside attention chunks) ----
    fillers = []
    popped = [0]
    allowed = [0]

    def pop_filler():
        if fillers and popped[0] < allowed[0]:
            fillers.pop(0)()
            popped[0] += 1

    def force_to(n):
        while popped[0] < n and fillers:
            fillers.pop(0)()
            popped[0] += 1

    # stage A: first chunk of pair-0 QKV, then attention starts
    qk_chunk(0, 0)
    v_chunk(0)
    # filler order: remaining pair-0 qkv, then qk for pairs 1-3, then outproj
    for sc in range(1, NSC):
        fillers.append(lambda s=sc: qk_chunk(0, s))
        fillers.append(lambda s=sc: v_chunk(s))
    for pp in range(1, NPAIR):
        fillers += [lambda p=pp, s=sc: qk_chunk(p, s) for sc in range(NSC)]
    # prereq: min filler-pops before emitting chunk (p, j)
    prereq = {}
    for j in range(1, NSC):
        prereq[(0, j)] = 2 * j  # qk(0,j) at idx 2j-2, v(j) at 2j-1
    for pp in range(1, NPAIR):
        for j in range(NSC):
            prereq[(pp, j)] = 6 + 4 * (pp - 1) + j + 1
    # pacing: max pops allowed by start of chunk (p, j) (baseline-style:
    # pair p runs pair p+1's qk; pair 3 additionally runs outproj phase A')
    pace = {(0, 0): 2, (0, 1): 4, (0, 2): 6, (0, 3): 7,
            (1, 0): 8, (1, 1): 9, (1, 2): 10, (1, 3): 11,
            (2, 0): 12, (2, 1): 13, (2, 2): 14, (2, 3): 15,
            (3, 0): 20, (3, 1): 26, (3, 2): 32, (3, 3): 34}

    att3 = None
    for p in range(NPAIR):
        att = [att_pool.tile([P, S], BF16, tag=f"att{h}", name=f"att{p}_{h}")
               for h in range(2)]
        if p == 3:
            att3 = att
            fillers.extend([lambda qt=qt: outproj_a(qt) for qt in range(S // P)])
        norm_prev = None
        for j in range(NSC):
            force_to(prereq.get((p, j), 0))
            allowed[0] = pace[(p, j)]
            norm_j = attention_chunk(p, j, att, pop_filler)
            if norm_prev is not None:
                norm_prev()
                if p == 3 and j == 2:
                    gather3([0, 1], agout3a)    # chunks 0+1 (both normalized)
                elif p == 3 and j == 3:
                    gather3([2], agout3b)       # chunk 2
            norm_prev = norm_j
        norm_prev()
        if p == 3:
            phase_b(0)
            force_to(10 ** 9)
        if p < 3:
            nc.sync.dma_start(agin[p, 0:DK], att[0][DK:P, :])
            nc.sync.dma_start(agin[p, DK:P], att[1][DK:P, :])
        if p == 1:
            nc.gpsimd.collective_compute(
                "AllGather", mybir.AluOpType.bypass, replica_groups=groups,
                ins=[agin[0:2].opt()], outs=[agout0[:].opt()])
            # i-block i: g_src, pr = divmod(i, 4); pairs 0-1: pr in {0,1}
            for i in [0, 1, 4, 5]:
                nc.sync.dma_start(agt[i][:], agout0[i // NPAIR, i % NPAIR])
        if p == 2:
            nc.gpsimd.collective_compute(
                "AllGather", mybir.AluOpType.bypass, replica_groups=groups,
                ins=[agin[2:3].opt()], outs=[agout1[:].opt()])
            for i in [2, 6]:
                nc.sync.dma_start(agt[i][:], agout1[i // NPAIR, 0])
        if p == 3:
            gather3([3], agout3c)       # chunk 3 (tail)
            phase_b(1)
            phase_b(2)

    phase_b(3)

    for cm in reversed(ctxs):
        cm.__exit__(None, None, None)


def _prep_in_maps(x, Wq, bq, Wk, bk, Wv, bv, Wo, bo):
    bf16 = ml_dtypes.bfloat16
    in_maps = []
    mask = np.zeros((4, P, SC), dtype=bf16)
    for r in range(4):
        k_idx = np.arange(P)[:, None]
        q_idx = np.arange(SC)[None, :]
        mask[r] = (q_idx >= P * r + k_idx).astype(bf16)
    for c in range(8):
        b, g = divmod(c, 2)
        dsl = slice(g * DG, (g + 1) * DG)
        xT = np.ascontiguousarray(x[b].T).astype(bf16)
        wqT = np.ascontiguousarray(Wq[dsl].T).astype(bf16)
        wkT = np.ascontiguousarray(Wk[dsl].T).astype(bf16)
        in_maps.append({
            "xTt": np.ascontiguousarray(
                xT.reshape(NI, P, NSC, SC).transpose(2, 1, 0, 3)),
            "wqt": np.ascontiguousarray(
                wqT.reshape(NI, P, NPAIR, P).transpose(2, 1, 0, 3)),
            "wkt": np.ascontiguousarray(
                wkT.reshape(NI, P, NPAIR, P).transpose(2, 1, 0, 3)),
            "wvt": np.ascontiguousarray(
                Wv[dsl].T.astype(bf16).reshape(NI, P, DG).transpose(1, 0, 2)),
            "wot": np.ascontiguousarray(
                Wo[dsl].T.astype(bf16).reshape(NI, P, DG).transpose(1, 0, 2)),
            "bq": np.ascontiguousarray(bq[dsl].reshape(NPAIR, P).T.astype(np.float32)),
            "bk": np.ascontiguousarray(bk[dsl].reshape(NPAIR, P).T.astype(np.float32)),
            "bv_bc": np.broadcast_to(bv[dsl].astype(np.float32), (P, DG)).copy(),
            "bo_bc": np.broadcast_to(bo[dsl].astype(np.float32), (P, DG)).copy(),
            "masks": np.ascontiguousarray(mask.transpose(1, 0, 2)),
        })
    return in_maps


def kernel(x, Wq, bq, Wk, bk, Wv, bv, Wo, bo, _trace=False, _trace_kwargs=None):
    x, Wq, bq, Wk, bk = map(np.asarray, (x, Wq, bq, Wk, bk))
    Wv, bv, Wo, bo = map(np.asarray, (Wv, bv, Wo, bo))
    if "nc" not in _cache:
        _cache["nc"] = _build()
    nc = _cache["nc"]
    in_maps = _prep_in_maps(x, Wq, bq, Wk, bk, Wv, bv, Wo, bo)
    res = bass_utils.run_bass_kernel_spmd(
        nc, in_maps, core_ids=list(range(8)), trace=_trace,
        **(_trace_kwargs or {}))
    _cache["last_result"] = res
    out = np.empty((B, S, D), dtype=np.float32)
    for c in range(8):
        b, g = divmod(c, 2)
        out[b, :, g * DG:(g + 1) * DG] = res.results[c]["out"]
    return out


# revision 14
# speedup vs baseline: 1.0112x; 1.0112x over previous
"""Causal multi-head attention on 8 TRN2 NeuronCores.

Sharding: core c = (batch b=c//2, head-group g=c%2). Each core computes QKV
projections + causal attention for its 8 heads over the full sequence of its
batch; 2-rank AllGathers (pair shares a batch) exchange attention outputs;
each core then runs the output projection for its half of the output columns.

v2 schedule notes:
  - host inputs re-laid-out so every DMA tile is one contiguous DRAM block;
    DMA waves ordered by first-use time (pair-0 qk weights + x chunk 0 first).
  - attention(0,0) starts right after qk(0,0)+v(0); remaining QKV chunks are
    PE fillers inside earlier attention chunks (global filler queue).
  - v_aug packed [P, 8, 65] (ones col + 64 v cols, no zero pad); softmax
    denominator comes from the ones row of the AV matmul (psum row 0,
    values rows 1:65).
  - exp pipeline: pending AV queue depth 5, 7 et buffers.
  - output projection: blocks [0,1,4,5,2,6] pre-accumulated into bf16 partial
    during pair-3 attention (phase A'); pair-3 attention outputs are gathered
    per q-chunk j; phase B(j) (blocks 3,7 + partial) pipelines behind each
    gather so only gather(3)+B(3) remain on the tail.
"""

import numpy as np
import ml_dtypes

import concourse.bass as bass
import concourse.mybir as mybir
import concourse.tile as tile
from concourse import bacc
from concourse import bass_utils

BF16 = mybir.dt.bfloat16
F32 = mybir.dt.float32

B, S, D = 4, 2048, 1024
H, DK = 16, 64
HPG = 8          # heads per group (per core)
DG = HPG * DK    # 512, d-range per core
NPAIR = 4        # head pairs per core
SC = 512         # sequence chunk (matmul free dim)
NSC = S // SC    # 4
KB = 128         # key block
NKB = S // KB    # 16
P = 128
NI = D // P      # 8
VW = DK + 1      # 65: ones col + v cols per head

_cache = {}


def _build():
    nc = bacc.Bacc("TRN2", target_bir_lowering=False, debug=False, num_devices=8)

    xTt = nc.dram_tensor("xTt", [NSC, P, NI, SC], BF16, kind="ExternalInput")
    wqt = nc.dram_tensor("wqt", [NPAIR, P, NI, P], BF16, kind="ExternalInput")
    wkt = nc.dram_tensor("wkt", [NPAIR, P, NI, P], BF16, kind="ExternalInput")
    wvt = nc.dram_tensor("wvt", [P, NI, DG], BF16, kind="ExternalInput")
    wot = nc.dram_tensor("wot", [P, NI, DG], BF16, kind="ExternalInput")
    bq = nc.dram_tensor("bq", [P, NPAIR], F32, kind="ExternalInput")
    bk = nc.dram_tensor("bk", [P, NPAIR], F32, kind="ExternalInput")
    bv_bc = nc.dram_tensor("bv_bc", [P, DG], F32, kind="ExternalInput")
    bo_bc = nc.dram_tensor("bo_bc", [P, DG], F32, kind="ExternalInput")
    masks = nc.dram_tensor("masks", [P, 4, SC], BF16, kind="ExternalInput")
    out = nc.dram_tensor("out", [S, DG], F32, kind="ExternalOutput")

    with tile.TileContext(nc) as tc:
        _emit(nc, tc, xTt, wqt, wkt, wvt, wot, bq, bk, bv_bc, bo_bc, masks, out)
    nc.compile()
    return nc


def _emit(nc, tc, xTt, wqt, wkt, wvt, wot, bq, bk, bv_bc, bo_bc, masks, out):
    ctxs = []

    def pool(name, bufs, space="SBUF"):
        cm = tc.tile_pool(name=name, bufs=bufs, space=space)
        p = cm.__enter__()
        ctxs.append(cm)
        return p

    const = pool("const", 1)
    dram = pool("dram", 1, space="DRAM")
    qk_pool = pool("qk", 2)
    att_pool = pool("att", 2)
    exp_pool = pool("exp", 5)
    small = pool("small", 3)
    out_pool = pool("outp", 3)
    ps_qk = pool("ps_qk", 2, space="PSUM")
    ps_sc = pool("ps_sc", 2, space="PSUM")
    ps_av = pool("ps_av", 2, space="PSUM")

    # ---- constants / weights, DMA waves ordered by first-use time ----
    xt = const.tile([P, NI, S], BF16, name="xt")
    wq = const.tile([P, NI, DG], BF16, name="wq")
    wk = const.tile([P, NI, DG], BF16, name="wk")
    wv = const.tile([P, NI, DG], BF16, name="wv")
    # wo aliases wv's storage (wv dead after the last v_chunk filler)
    wo = const.tile([P, NI, DG], BF16, tag="wv", name="wo")
    # wave 1: x chunk 0, pair-0 qk weight columns, small tensors
    nc.sync.dma_start(xt[:, :, 0:SC], xTt[0])
    nc.sync.dma_start(wq[:, :, 0:P], wqt[0])
    nc.sync.dma_start(wk[:, :, 0:P], wkt[0])
    bq_t = const.tile([P, NPAIR], F32, name="bq_t")
    bk_t = const.tile([P, NPAIR], F32, name="bk_t")
    bv_t = const.tile([P, DG], F32, name="bv_t")
    nc.sync.dma_start(bq_t[:], bq[:])
    nc.sync.dma_start(bk_t[:], bk[:])
    nc.sync.dma_start(bv_t[:], bv_bc[:])
    mask_t = const.tile([P, 4, SC], BF16, name="mask_t")
    nc.sync.dma_start(mask_t[:], masks[:])
    # wave 2: v weights (needed by v_chunk(0))
    nc.sync.dma_start(wv[:], wvt[:])
    # wave 3: remaining x chunks
    for sc in range(1, NSC):
        nc.sync.dma_start(xt[:, :, SC * sc:SC * (sc + 1)], xTt[sc])
    # wave 4: qk weights pairs 1-3
    for p in range(1, NPAIR):
        nc.sync.dma_start(wq[:, :, P * p:P * (p + 1)], wqt[p])
        nc.sync.dma_start(wk[:, :, P * p:P * (p + 1)], wkt[p])
    # wave 5: output projection weights + bias
    nc.sync.dma_start(wo[:], wot[:])
    bo_t = const.tile([P, DG], F32, name="bo_t")
    nc.sync.dma_start(bo_t[:], bo_bc[:])

    # v_aug[t]: [128, 8, 128]; per head h: col 0 = ones (softmax denominator
    # row), cols 1:64 = zeros (psum partition alignment pad), cols 64:128 = v.
    # Built via memsets (no host DMA).
    v_aug = [const.tile([P, HPG, P], BF16, name=f"va{t}", tag=f"va{t}")
             for t in range(NKB)]
    for t in range(NKB):
        nc.gpsimd.memset(v_aug[t][:, :, 0:DK], 0.0)
        nc.gpsimd.memset(v_aug[t][:, :, 0:1], 1.0)

    # DRAM bounce buffers for the pairwise AllGathers
    agin = dram.tile([3, P, S], BF16, name="agin")          # pairs 0-2
    agout0 = dram.tile([2, 2, P, S], BF16, name="agout0")   # pairs 0-1
    agout1 = dram.tile([2, 1, P, S], BF16, name="agout1")   # pair 2
    agin3 = dram.tile([NSC, P, SC], BF16, name="agin3")     # pair 3, per chunk
    agout3a = dram.tile([2, 2, P, SC], BF16, name="agout3a")  # chunks 0+1
    agout3b = dram.tile([2, P, SC], BF16, name="agout3b")     # chunk 2
    agout3c = dram.tile([2, P, SC], BF16, name="agout3c")     # chunk 3

    groups = [[0, 1], [2, 3], [4, 5], [6, 7]]

    qT_pair = [qk_pool.tile([P, S], BF16, tag="qT", name=f"qTp{pp}")
               for pp in range(NPAIR)]
    kT_pair = [qk_pool.tile([P, S], BF16, tag="kT", name=f"kTp{pp}")
               for pp in range(NPAIR)]

    def qk_chunk(p, sc):
        """q/k projections for pair p, seq chunk sc."""
        ssl = slice(SC * sc, SC * (sc + 1))
        ps_q = ps_qk.tile([P, SC], F32, tag="psqk", name=f"psq{p}_{sc}")
        for i in range(NI):
            nc.tensor.matmul(ps_q[:], lhsT=wq[:, i, P * p:P * (p + 1)],
                             rhs=xt[:, i, ssl], start=(i == 0), stop=(i == 7))
        nc.vector.tensor_add(qT_pair[p][:, ssl], ps_q[:],
                             bq_t[:, p:p + 1].to_broadcast((P, SC)))
        ps_k = ps_qk.tile([P, SC], F32, tag="psqk", name=f"psk{p}_{sc}")
        for i in range(NI):
            nc.tensor.matmul(ps_k[:], lhsT=wk[:, i, P * p:P * (p + 1)],
                             rhs=xt[:, i, ssl], start=(i == 0), stop=(i == 7))
        nc.vector.tensor_add(kT_pair[p][:, ssl], ps_k[:],
                             bk_t[:, p:p + 1].to_broadcast((P, SC)))

    def v_chunk(sc):
        """v projection for seq chunk sc (all 8 heads), into v_aug tiles."""
        for st in range(4):
            t = 4 * sc + st
            ps_v = ps_qk.tile([P, DG], F32, tag="psqk", name=f"psv{sc}_{st}")
            for i in range(NI):
                nc.tensor.matmul(ps_v[:], lhsT=xt[:, i, P * t:P * (t + 1)],
                                 rhs=wv[:, i, :], start=(i == 0), stop=(i == 7))
            va3 = v_aug[t]
            nc.vector.tensor_add(va3[:, :, DK:P],
                                 ps_v[:].rearrange("p (h c) -> p h c", c=DK),
                                 bv_t[:].rearrange("p (h c) -> p h c", c=DK))

    def attention_chunk(p, j, att, pop_filler):
        """Causal attention for head pair p, q chunk j. Both heads row-packed
        into one wide psum; one wide exp; diag blocks first."""
        avs = [ps_av.tile([P, SC], F32, tag="av", name=f"av{p}_{j}_{h}")
               for h in range(2)]
        nkb = 4 * (j + 1)
        kbs = list(range(4 * j, nkb)) + list(range(0, 4 * j))  # diag first
        pending = []
        issued = [0]

        def issue_av(item):
            kb, qlo, et = item
            et3 = et.rearrange("p (h w) -> p h w", w=SC)
            for h in range(2):
                hh = 2 * p + h
                nc.tensor.matmul(avs[h][:, qlo:], lhsT=v_aug[kb][:, hh, :],
                                 rhs=et3[:, h, qlo:],
                                 start=(issued[0] == 0),
                                 stop=(issued[0] == nkb - 1))
            issued[0] += 1

        for n, kb in enumerate(kbs):
            r = kb - 4 * j  # >= 0 on diagonal blocks
            qlo = P * r if r >= 0 else 0
            ps_s = ps_sc.tile([P, 2 * SC], F32, tag="sc", name=f"pss{p}_{j}_{kb}")
            for h in range(2):
                hb = slice(DK * h, DK * (h + 1))
                nc.tensor.matmul(
                    ps_s[:, SC * h + qlo:SC * (h + 1)],
                    lhsT=kT_pair[p][hb, P * kb:P * (kb + 1)],
                    rhs=qT_pair[p][hb, SC * j + qlo:SC * (j + 1)],
                    start=True, stop=True)
            et = exp_pool.tile([P, 2 * SC], BF16, tag="exp", name=f"et{p}_{j}_{kb}")
            ps3 = ps_s.rearrange("p (h w) -> p h w", w=SC)
            et3 = et.rearrange("p (h w) -> p h w", w=SC)
            nc.scalar.activation(et3[:, :, qlo:], ps3[:, :, qlo:],
                                 mybir.ActivationFunctionType.Exp, scale=0.125)
            if r >= 0:
                nc.vector.tensor_mul(
                    et3[:, :, qlo:], et3[:, :, qlo:],
                    mask_t[:, r:r + 1, qlo:].to_broadcast((P, 2, SC - qlo)))
            pending.append((kb, qlo, et))
            while len(pending) > 4:
                issue_av(pending.pop(0))
            if n % 2 == 1:
                pop_filler()
        while pending:
            issue_av(pending.pop(0))

        # eagerly free the av psum slots: reciprocal of the sums row plus a
        # copy of the value rows to SBUF; the gpsimd broadcast + multiply are
        # lazy and may be delayed by collectives on the gpsimd queue without
        # stalling the PSUM pipeline.
        sums_l = []
        avc_l = []
        for h in range(2):
            sums = small.tile([1, SC], F32, tag="sums", name=f"sums{p}_{j}_{h}")
            nc.vector.reciprocal_approx_fast(sums[0:1, :], avs[h][0:1, :])
            sums_l.append(sums)
            avc = small.tile([P, SC], F32, tag=f"avc{h}", name=f"avc{p}_{j}_{h}",
                             bufs=2)
            nc.vector.tensor_copy(avc[DK:P, :], avs[h][DK:P, :])
            avc_l.append(avc)

        def normalize():
            for h in range(2):
                rb = small.tile([P, SC], F32, tag="rb", name=f"rb{p}_{j}_{h}")
                nc.gpsimd.partition_broadcast(rb[:], sums_l[h][0:1, :])
                nc.vector.tensor_mul(att[h][DK:P, SC * j:SC * (j + 1)],
                                     avc_l[h][DK:P, :], rb[DK:P, :])
        return normalize

    # gathered attention outputs (out-proj lhsT), one [P, S] tile per i-block
    agt = [const.tile([P, S], BF16, name=f"agt{i}", tag=f"agt{i}")
           for i in range(NI)]
    # out-proj partials (blocks 0,1,4,5,2,6), bf16, bo included
    part_lo = const.tile([P, NI, SC], BF16, tag="wq", name="part_lo")
    part_hi = const.tile([P, NI, SC], BF16, tag="wk", name="part_hi")

    def agt_lhsT(i, qt):
        return agt[i][:, P * qt:P * (qt + 1)]

    def part_slice(qt):
        t = part_lo if qt < 8 else part_hi
        return t[:, qt % 8, :]

    def outproj_a(qt):
        """Phase A': accumulate blocks 0,1,4,5,2,6 for q-tile qt -> bf16
        partial (bias included)."""
        ps_o = ps_qk.tile([P, DG], F32, tag="psqk", name=f"psoa{qt}")
        for n, i in enumerate([0, 1, 4, 5, 2, 6]):
            nc.tensor.matmul(ps_o[:], lhsT=agt_lhsT(i, qt),
                             rhs=wo[:, i, :], start=(n == 0), stop=(n == 5))
        nc.vector.tensor_add(part_slice(qt), ps_o[:], bo_t[:])

    def phase_b(j):
        """Phase B(j): blocks 3,7 + phase-A partial -> out, for q-tiles of
        gather chunk j."""
        for qt in range(4 * j, 4 * (j + 1)):
            ps_o = ps_qk.tile([P, DG], F32, tag="psqk", name=f"psob{qt}")
            for n, i in enumerate([3, 7]):
                nc.tensor.matmul(ps_o[:], lhsT=agt_lhsT(i, qt),
                                 rhs=wo[:, i, :], start=(n == 0), stop=(n == 1))
            ot = out_pool.tile([P, DG], F32, tag="ot", name=f"ot{qt}")
            nc.vector.tensor_add(ot[:], ps_o[:], part_slice(qt))
            q = nc.sync if qt % 2 == 0 else nc.scalar
            q.dma_start(out[P * qt:P * (qt + 1), :], ot[:])

    def gather3(js, agout):
        """Pair-3 partial gather of chunks js (contiguous run) for both heads
        -> agt[3], agt[7] column slices."""
        for jj in js:
            jsl = slice(SC * jj, SC * (jj + 1))
            nc.sync.dma_start(agin3[jj, 0:DK], att3[0][DK:P, jsl])
            nc.sync.dma_start(agin3[jj, DK:P], att3[1][DK:P, jsl])
        nc.gpsimd.collective_compute(
            "AllGather", mybir.AluOpType.bypass, replica_groups=groups,
            ins=[agin3[js[0]:js[-1] + 1].opt()], outs=[agout[:].opt()])
        for n, jj in enumerate(js):
            jsl = slice(SC * jj, SC * (jj + 1))
            if len(js) > 1:
                nc.sync.dma_start(agt[3][:, jsl], agout[0, n])
                nc.sync.dma_start(agt[7][:, jsl], agout[1, n])
            else:
                nc.sync.dma_start(agt[3][:, jsl], agout[0])
                nc.sync.dma_start(agt[7][:, jsl], agout[1])

    # ---- global filler queue (PE work injected inside attention chunks) ----
    fillers = []
    popped = [0]
    allowed = [0]

    def pop_filler():
        if fillers and popped[0] < allowed[0]:
            fillers.pop(0)()
            popped[0] += 1

    def force_to(n):
        while popped[0] < n and fillers:
            fillers.pop(0)()
            popped[0] += 1

    # stage A: first chunk of pair-0 QKV, then attention starts
    qk_chunk(0, 0)
    v_chunk(0)
    # filler order: remaining pair-0 qkv, then qk for pairs 1-3, then outproj
    for sc in range(1, NSC):
        fillers.append(lambda s=sc: qk_chunk(0, s))
        fillers.append(lambda s=sc: v_chunk(s))
    for pp in range(1, NPAIR):
        fillers += [lambda p=pp, s=sc: qk_chunk(p, s) for sc in range(NSC)]
    # prereq: min filler-pops before emitting chunk (p, j)
    prereq = {}
    for j in range(1, NSC):
        prereq[(0, j)] = 2 * j  # qk(0,j) at idx 2j-2, v(j) at 2j-1
    for pp in range(1, NPAIR):
        for j in range(NSC):
            prereq[(pp, j)] = 6 + 4 * (pp - 1) + j + 1
    # pacing: max pops allowed by start of chunk (p, j) (baseline-style:
    # pair p runs pair p+1's qk; pair 3 additionally runs outproj phase A')
    pace = {(0, 0): 2, (0, 1): 4, (0, 2): 6, (0, 3): 7,
            (1, 0): 8, (1, 1): 9, (1, 2): 10, (1, 3): 11,
            (2, 0): 12, (2, 1): 13, (2, 2): 14, (2, 3): 15,
            (3, 0): 20, (3, 1): 26, (3, 2): 32, (3, 3): 34}

    att3 = None
    for p in range(NPAIR):
        att = [att_pool.tile([P, S], BF16, tag=f"att{h}", name=f"att{p}_{h}")
               for h in range(2)]
        if p == 3:
            att3 = att
            fillers.extend([lambda qt=qt: outproj_a(qt) for qt in range(S // P)])
        norm_prev = None
        for j in range(NSC):
            force_to(prereq.get((p, j), 0))
            allowed[0] = pace[(p, j)]
            norm_j = attention_chunk(p, j, att, pop_filler)
            if norm_prev is not None:
                norm_prev()
                if p == 3 and j == 2:
                    gather3([0, 1], agout3a)    # chunks 0+1 (both normalized)
                elif p == 3 and j == 3:
                    gather3([2], agout3b)       # chunk 2
            norm_prev = norm_j
        norm_prev()
        if p == 3:
            phase_b(0)
            force_to(10 ** 9)
        if p < 3:
            nc.sync.dma_start(agin[p, 0:DK], att[0][DK:P, :])
            nc.sync.dma_start(agin[p, DK:P], att[1][DK:P, :])
        if p == 1:
            nc.gpsimd.collective_compute(
                "AllGather", mybir.AluOpType.bypass, replica_groups=groups,
                ins=[agin[0:2].opt()], outs=[agout0[:].opt()])
            # i-block i: g_src, pr = divmod(i, 4); pairs 0-1: pr in {0,1}
            for i in [0, 1, 4, 5]:
                nc.sync.dma_start(agt[i][:], agout0[i // NPAIR, i % NPAIR])
        if p == 2:
            nc.gpsimd.collective_compute(
                "AllGather", mybir.AluOpType.bypass, replica_groups=groups,
                ins=[agin[2:3].opt()], outs=[agout1[:].opt()])
            for i in [2, 6]:
                nc.sync.dma_start(agt[i][:], agout1[i // NPAIR, 0])
        if p == 3:
            gather3([3], agout3c)       # chunk 3 (tail)
            phase_b(1)
            phase_b(2)

    phase_b(3)

    for cm in reversed(ctxs):
        cm.__exit__(None, None, None)


def _prep_in_maps(x, Wq, bq, Wk, bk, Wv, bv, Wo, bo):
    bf16 = ml_dtypes.bfloat16
    in_maps = []
    mask = np.zeros((4, P, SC), dtype=bf16)
    for r in range(4):
        k_idx = np.arange(P)[:, None]
        q_idx = np.arange(SC)[None, :]
        mask[r] = (q_idx >= P * r + k_idx).astype(bf16)
    for c in range(8):
        b, g = divmod(c, 2)
        dsl = slice(g * DG, (g + 1) * DG)
        xT = np.ascontiguousarray(x[b].T).astype(bf16)
        wqT = np.ascontiguousarray(Wq[dsl].T).astype(bf16)
        wkT = np.ascontiguousarray(Wk[dsl].T).astype(bf16)
        in_maps.append({
            "xTt": np.ascontiguousarray(
                xT.reshape(NI, P, NSC, SC).transpose(2, 1, 0, 3)),
            "wqt": np.ascontiguousarray(
                wqT.reshape(NI, P, NPAIR, P).transpose(2, 1, 0, 3)),
            "wkt": np.ascontiguousarray(
                wkT.reshape(NI, P, NPAIR, P).transpose(2, 1, 0, 3)),
            "wvt": np.ascontiguousarray(
                Wv[dsl].T.astype(bf16).reshape(NI, P, DG).transpose(1, 0, 2)),
            "wot": np.ascontiguousarray(
                Wo[dsl].T.astype(bf16).reshape(NI, P, DG).transpose(1, 0, 2)),
            "bq": np.ascontiguousarray(bq[dsl].reshape(NPAIR, P).T.astype(np.float32)),
            "bk": np.ascontiguousarray(bk[dsl].reshape(NPAIR, P).T.astype(np.float32)),
            "bv_bc": np.broadcast_to(bv[dsl].astype(np.float32), (P, DG)).copy(),
            "bo_bc": np.broadcast_to(bo[dsl].astype(np.float32), (P, DG)).copy(),
            "masks": np.ascontiguousarray(mask.transpose(1, 0, 2)),
        })
    return in_maps


def kernel(x, Wq, bq, Wk, bk, Wv, bv, Wo, bo, _trace=False, _trace_kwargs=None):
    x, Wq, bq, Wk, bk = map(np.asarray, (x, Wq, bq, Wk, bk))
    Wv, bv, Wo, bo = map(np.asarray, (Wv, bv, Wo, bo))
    if "nc" not in _cache:
        _cache["nc"] = _build()
    nc = _cache["nc"]
    in_maps = _prep_in_maps(x, Wq, bq, Wk, bk, Wv, bv, Wo, bo)
    res = bass_utils.run_bass_kernel_spmd(
        nc, in_maps, core_ids=list(range(8)), trace=_trace,
        **(_trace_kwargs or {}))
    _cache["last_result"] = res
    out = np.empty((B, S, D), dtype=np.float32)
    for c in range(8):
        b, g = divmod(c, 2)
        out[b, :, g * DG:(g + 1) * DG] = res.results[c]["out"]
    return out
